# revision 2
# baseline (speedup 1.0000x reference)
"""Probabilistic-circuit (einsum-network) forward pass, data-parallel over batch.

Contract: kernel(**inputs) takes FULL unsharded numpy inputs and returns the
FULL (B, 1, K) output. Internally the batch axis (B=2048) is sharded across
the 8 NeuronCores; all bookkeeping indices and per-fold weights are
replicated on every core (no cross-device traffic inside the traversal).
"""

import numpy as np

_LOG2PI = 0.9189385332046727
_NUM_LEVELS = 8


def _build_circuit(mu, log_sigma, in_scope_idx, fold_idxs, ws):
    import jax
    import jax.numpy as jnp

    mu = jnp.asarray(mu)
    log_sigma = jnp.asarray(log_sigma)
    inv_sigma = jnp.exp(-log_sigma)
    # Precompute softmax-normalized fold weights once (replicated, tiny).
    wps = [jax.nn.softmax(jnp.asarray(w), axis=-1) for w in ws]

    def circuit(xs):  # xs: (b_shard, 1, D)
        xg = jnp.transpose(xs[..., in_scope_idx], (2, 1, 0, 3))  # (D,1,b,1)
        xv = xg[:, 0, :, 0]  # (D, b)
        z = (xv[:, :, None] - mu[:, None, :]) * inv_sigma[:, None, :]
        out = -0.5 * z * z - log_sigma[:, None, :] - _LOG2PI  # (D, b, K)
        for fidx, wp in zip(fold_idxs, wps):
            h = out[fidx].sum(axis=1)  # (F, b, K)
            m = jnp.max(h, axis=-1, keepdims=True)
            out = jnp.log(jnp.einsum("fbk,fjk->fbj", jnp.exp(h - m), wp)) + m
        return jnp.transpose(out, (1, 0, 2))  # (b, 1, K)

    return circuit


def kernel(**inputs) -> np.ndarray:
    import jax

    x = np.asarray(inputs["x"])  # (2048, 1, 256) float32
    mu = np.asarray(inputs["mu"])  # (256, 64)
    log_sigma = np.asarray(inputs["log_sigma"])  # (256, 64)
    in_scope_idx = np.asarray(inputs["in_scope_idx"])  # (256, 1)
    fold_idxs = [np.asarray(inputs[f"fold_idx{l}"]) for l in range(1, _NUM_LEVELS + 1)]
    ws = [np.asarray(inputs[f"w{l}"]) for l in range(1, _NUM_LEVELS + 1)]

    B = x.shape[0]
    circuit = _build_circuit(mu, log_sigma, in_scope_idx, fold_idxs, ws)

    n_dev = min(8, jax.local_device_count())
    while n_dev > 1 and B % n_dev != 0:
        n_dev -= 1

    out = None
    try:
        if n_dev > 1:
            # Data-parallel over batch: shard x on B, replicate params.
            xsh = x.reshape(n_dev, B // n_dev, x.shape[1], x.shape[2])
            out = jax.pmap(circuit)(xsh)  # (n_dev, b, 1, K)
            out = np.asarray(out)
            out = out.reshape(B, out.shape[2], out.shape[3])
    except Exception:
        out = None
    if out is None:
        # Robust fallback: run the same computation on the host CPU backend.
        cpu = jax.devices("cpu")[0]
        with jax.default_device(cpu):
            out = np.asarray(jax.jit(circuit)(x))

    return out.astype(np.float32)


# revision 5
# speedup vs baseline: 14.3991x; 14.3991x over previous
"""Probabilistic-circuit (einsum-network) forward pass, data-parallel over batch.

Contract: kernel(**inputs) takes FULL unsharded numpy inputs and returns the
FULL (B, 1, K) output. Internally the batch axis (B=2048) is sharded across
the 8 NeuronCores; all bookkeeping indices and per-fold weights are
replicated on every core (no cross-device traffic inside the traversal).
"""

import numpy as np

_LOG2PI = 0.9189385332046727
_NUM_LEVELS = 8


def _build_circuit(mu, log_sigma, in_scope_idx, fold_idxs, ws):
    import jax
    import jax.numpy as jnp

    mu = jnp.asarray(mu)
    log_sigma = jnp.asarray(log_sigma)
    inv_sigma = jnp.exp(-log_sigma)
    # Precompute softmax-normalized fold weights once (replicated, tiny).
    wps = [jax.nn.softmax(jnp.asarray(w), axis=-1) for w in ws]

    def circuit(xs):  # xs: (b_shard, 1, D)
        xg = jnp.transpose(xs[..., in_scope_idx], (2, 1, 0, 3))  # (D,1,b,1)
        xv = xg[:, 0, :, 0]  # (D, b)
        z = (xv[:, :, None] - mu[:, None, :]) * inv_sigma[:, None, :]
        out = -0.5 * z * z - log_sigma[:, None, :] - _LOG2PI  # (D, b, K)
        for fidx, wp in zip(fold_idxs, wps):
            h = out[fidx].sum(axis=1)  # (F, b, K)
            m = jnp.max(h, axis=-1, keepdims=True)
            out = jnp.log(jnp.einsum("fbk,fjk->fbj", jnp.exp(h - m), wp)) + m
        return jnp.transpose(out, (1, 0, 2))  # (b, 1, K)

    return circuit


_FN_CACHE = {}


def kernel(**inputs) -> np.ndarray:
    import hashlib

    import jax

    x = np.asarray(inputs["x"])  # (2048, 1, 256) float32
    mu = np.asarray(inputs["mu"])  # (256, 64)
    log_sigma = np.asarray(inputs["log_sigma"])  # (256, 64)
    in_scope_idx = np.asarray(inputs["in_scope_idx"])  # (256, 1)
    fold_idxs = [np.asarray(inputs[f"fold_idx{l}"]) for l in range(1, _NUM_LEVELS + 1)]
    ws = [np.asarray(inputs[f"w{l}"]) for l in range(1, _NUM_LEVELS + 1)]

    B = x.shape[0]

    # The compiled executable is specialized on the replicated parameters
    # (indices + weights); cache it so repeat calls skip trace/compile.
    h = hashlib.sha1()
    for a in [mu, log_sigma, in_scope_idx, *fold_idxs, *ws]:
        h.update(np.ascontiguousarray(a).tobytes())
    key = (x.shape, h.hexdigest())
    entry = _FN_CACHE.get(key)
    if entry is None:
        c = _build_circuit(mu, log_sigma, in_scope_idx, fold_idxs, ws)
        entry = {"circuit": c, "pmap": jax.pmap(c), "jit": jax.jit(c)}
        _FN_CACHE[key] = entry
    circuit = entry["circuit"]

    n_dev = min(8, jax.local_device_count())
    while n_dev > 1 and B % n_dev != 0:
        n_dev -= 1

    out = None
    try:
        if n_dev > 1:
            # Data-parallel over batch: shard x on B, replicate params.
            xsh = x.reshape(n_dev, B // n_dev, x.shape[1], x.shape[2])
            out = entry["pmap"](xsh)  # (n_dev, b, 1, K)
            out = np.asarray(out)
            out = out.reshape(B, out.shape[2], out.shape[3])
    except Exception:
        out = None
    if out is None:
        # Robust fallback: run the same computation on the host CPU backend.
        cpu = jax.devices("cpu")[0]
        with jax.default_device(cpu):
            out = np.asarray(entry["jit"](x))

    return out.astype(np.float32)


# revision 6
# speedup vs baseline: 14.7208x; 1.0223x over previous
"""Probabilistic-circuit (einsum-network) forward pass, data-parallel over batch.

Contract: kernel(**inputs) takes FULL unsharded numpy inputs and returns the
FULL (B, 1, K) output. Internally the batch axis (B=2048) is sharded across
the 8 NeuronCores; all bookkeeping indices and per-fold weights are
replicated on every core (no cross-device traffic inside the traversal).

The fold bookkeeping (in_scope_idx, fold_idx1..8) is resolved on the host
into a single permutation cascade: level-l folds are laid out so every
level's gather is the adjacent pair (2g, 2g+1). The device graph then
contains no gathers at all — just reshapes, elementwise ops, and matmuls.
"""

import numpy as np

_LOG2PI = 0.9189385332046727
_NUM_LEVELS = 8


def _fold_orders(fold_idxs):
    """fold_orders[l] = original fold index at position p of level l, chosen
    so the children of position p at level l sit at positions (2p, 2p+1) of
    level l-1."""
    orders = [None] * (_NUM_LEVELS + 1)
    orders[_NUM_LEVELS] = np.zeros(1, dtype=np.int64)
    for l in range(_NUM_LEVELS, 0, -1):
        fo = orders[l]
        fidx = fold_idxs[l - 1]
        prev = np.empty(2 * len(fo), dtype=np.int64)
        prev[0::2] = fidx[fo, 0]
        prev[1::2] = fidx[fo, 1]
        orders[l - 1] = prev
    return orders


def _build_circuit(mu_p, ls_p, ws_p):
    import jax
    import jax.numpy as jnp

    mu_p = jnp.asarray(mu_p)
    ls_p = jnp.asarray(ls_p)
    inv_sigma = jnp.exp(-ls_p)
    wps = [jax.nn.softmax(jnp.asarray(w), axis=-1) for w in ws_p]

    def circuit(xg):  # xg: (b_shard, D) already scope-permuted
        xv = xg.T  # (D, b)
        z = (xv[:, :, None] - mu_p[:, None, :]) * inv_sigma[:, None, :]
        out = -0.5 * z * z - ls_p[:, None, :] - _LOG2PI  # (D, b, K)
        for wp in wps:
            F2, b, K = out.shape
            h = out.reshape(F2 // 2, 2, b, K).sum(axis=1)  # adjacent pairs
            m = jnp.max(h, axis=-1, keepdims=True)
            out = jnp.log(jnp.einsum("fbk,fjk->fbj", jnp.exp(h - m), wp)) + m
        return jnp.transpose(out, (1, 0, 2))  # (b, 1, K)

    return circuit


_FN_CACHE = {}


def kernel(**inputs) -> np.ndarray:
    import hashlib

    import jax

    x = np.asarray(inputs["x"])  # (2048, 1, 256) float32
    mu = np.asarray(inputs["mu"])  # (256, 64)
    log_sigma = np.asarray(inputs["log_sigma"])  # (256, 64)
    in_scope_idx = np.asarray(inputs["in_scope_idx"])  # (256, 1)
    fold_idxs = [np.asarray(inputs[f"fold_idx{l}"]) for l in range(1, _NUM_LEVELS + 1)]
    ws = [np.asarray(inputs[f"w{l}"]) for l in range(1, _NUM_LEVELS + 1)]

    B = x.shape[0]

    # Host-side bookkeeping: permutation cascade -> adjacent-pair layout.
    orders = _fold_orders(fold_idxs)
    ord0 = orders[0]
    scope_p = in_scope_idx[ord0, 0]  # variable index per position
    mu_p = mu[ord0]
    ls_p = log_sigma[ord0]
    ws_p = [ws[l - 1][orders[l]] for l in range(1, _NUM_LEVELS + 1)]

    # The compiled executable is specialized on the replicated parameters
    # (indices + weights); cache it so repeat calls skip trace/compile.
    h = hashlib.sha1()
    for a in [mu_p, ls_p, scope_p, *ws_p]:
        h.update(np.ascontiguousarray(a).tobytes())
    key = (x.shape, h.hexdigest())
    entry = _FN_CACHE.get(key)
    if entry is None:
        c = _build_circuit(mu_p, ls_p, ws_p)
        entry = {"pmap": jax.pmap(c), "jit": jax.jit(c)}
        _FN_CACHE[key] = entry

    # Input-layer scope gather done on host as part of sharding.
    xg = np.ascontiguousarray(x[:, 0, :][:, scope_p])  # (B, D)

    n_dev = min(8, jax.local_device_count())
    while n_dev > 1 and B % n_dev != 0:
        n_dev -= 1

    out = None
    try:
        if n_dev > 1:
            # Data-parallel over batch: shard xg on B, replicate params.
            xsh = xg.reshape(n_dev, B // n_dev, xg.shape[1])
            out = entry["pmap"](xsh)  # (n_dev, b, 1, K)
            out = np.asarray(out)
            out = out.reshape(B, out.shape[2], out.shape[3])
    except Exception:
        out = None
    if out is None:
        # Robust fallback: run the same computation on the host CPU backend.
        cpu = jax.devices("cpu")[0]
        with jax.default_device(cpu):
            out = np.asarray(entry["jit"](xg))

    return out.astype(np.float32)
